# revision 64
# baseline (speedup 1.0000x reference)
"""Trainium2 Bass kernel for nn_HeatmapEncoder.

Math per (b, s, c) and per coordinate set (gaze, hand):
    g = exp(-((gx-cx)^2 + (gy-cy)^2) / (2 sigma^2))   on a 336x336 grid
    g = g / (sum(g) + eps)            (zeroed when cx+cy <= 0)
    unified = g_gaze + g_hand
    out = unified / (max(unified) + eps)

Each unified map is rank-2 (separable Gaussian).  The host marshals the
coordinate inputs into the O(N) 1-D pieces: normalized x/y factor rows
in fp8 hi/lo form (the hi/lo split keeps matmul precision at ~2^-8),
and the per-map peak scale 1/(max+eps).  The peak of a two-Gaussian
equal-sigma mixture lies on the segment between the centers, so the
host finds it with a vectorized Newton iteration plus exact grid-point
evaluation of a small candidate set (measured underestimate < 2e-5).

The device runs the O(N^2) work: per map, four fp8 DoubleRow matmuls
(K=6 packed as 3 partitions x 2 set-pairs; the middle channel is split
at the PSUM bank boundary so 2-bank tiles give 4 pipeline slots), then
a scaled drain of PSUM split between ACT and DVE (~600 ns each) that
applies the host peak scale and writes fp16, and a grouped 4-map output
DMA with 8 KB descriptors.  Output DRAM layout is partition-major
[112, 32, 1008] fp16; the host reassembles to [B, S, C, 336, 336] fp32.

Sharding: pure data parallel over batch B=8 across the 8 cores.
"""

import functools
from contextlib import ExitStack

import numpy as np

try:
    import concourse.bass as bass
except ImportError:  # pragma: no cover
    import sys

    sys.path.insert(0, "/opt/trn_rl_repo")
    import concourse.bass as bass

import concourse.tile as tile
from concourse import bacc, mybir
from concourse.bass_utils import run_bass_kernel_spmd

H = W = 336
P = 112  # partitions per y-chunk; y = 3*p + c  (c in 0..2)
NCH = 3
NW = NCH * W
S_DIM, C_DIM = 8, 4
NMAPS = S_DIM * C_DIM  # 32 maps per core
NR = 2 * NMAPS  # 64 coordinate rows (map-major, gaze/hand interleaved)
NB = 8  # free blocks in the aligned factor layout (map j = 4*b + q)
N_CORES = 8
SIGMA = 10.0 / 336.0
EXP_SCALE = -1.0 / (2.0 * SIGMA * SIGMA)
EPS = 1e-6
DELTA = 1e-9
GROUP = 4
XS = 560  # ACT handles drain cols [0, XS), DVE handles [XS, NW)

F32 = mybir.dt.float32
FP16 = mybir.dt.float16
FP8 = mybir.dt.float8e4
F8NP = mybir.dt.np(FP8)
AF = mybir.ActivationFunctionType

# fp8 pre-scales keep both factor hi/lo terms in e4m3 normal range; the
# drain compensates through the host-computed peak scale
Y_GAIN, X_GAIN = 16.0, 256.0
DRAIN_SCALE = 1.0 / (Y_GAIN * X_GAIN)

GATHER_Q = ("sync", "scalar", "gpsimd")

_GRID64 = (np.arange(W, dtype=np.float64) / (W - 1)).astype(
    np.float32).astype(np.float64)


def _emit(nc, tc, ctx, fpl_in, rgc_in, out_t):
    const = ctx.enter_context(tc.tile_pool(name="const", bufs=1))
    ffac = ctx.enter_context(tc.tile_pool(name="ffac", bufs=1))
    ustage = ctx.enter_context(tc.tile_pool(name="ustage", bufs=5))
    pmap = ctx.enter_context(tc.tile_pool(name="pmap", bufs=4, space="PSUM"))

    # ACT table preload via dummy exp on a memset tile
    dum = const.tile([1, 16], F32, tag="dum")
    nc.vector.memset(dum[:], 0.0)
    dum2 = const.tile([1, 16], F32, tag="dum2")
    nc.scalar.activation(dum2[:], dum[:], AF.Exp, bias=0.0, scale=1.0)

    # per-q factor tiles gathered straight from the input planes.
    # DoubleRow layout: term u on partition 32q+u, set t in the pair slot.
    FYq = [ffac.tile([128, NB, 2, W], FP8, name=f"FY{q}", tag=f"fy{q}")
           for q in range(4)]
    FXq = [ffac.tile([128, NB, 2, W], FP8, name=f"FX{q}", tag=f"fx{q}")
           for q in range(4)]
    sav = fpl_in.ap().rearrange("u (b r) x -> u b r x", r=8)

    def gather(q):
        yq = getattr(nc, GATHER_Q[(2 * q) % 3])
        xq = getattr(nc, GATHER_Q[(2 * q + 1) % 3])
        # dest [part 32q+u, b, t, x]  <-  fpl[u, 8b+2q+t(+64 for x), x]
        yq.dma_start(FYq[q][32 * q:32 * q + 3, :, :, :],
                     sav[:, 0:8, 2 * q:2 * q + 2, :])
        xq.dma_start(FXq[q][32 * q:32 * q + 3, :, :, :],
                     sav[:, 8:16, 2 * q:2 * q + 2, :])

    # q0 gathers lead both queues — they gate the first matmul; the RGC
    # scales (first needed by the drains, ~5us later) queue behind them
    gather(0)
    RGC = const.tile([P, NMAPS], F32)
    nc.sync.dma_start(RGC[:], rgc_in.ap())
    for q in range(1, 4):
        gather(q)

    # per-channel output column ranges; c1 is split at the 512-column PSUM
    # bank boundary so no matmul output ever spans a bank (2-bank tiles)
    MM_COLS = ((0, 0, W), (336, 0, 176), (512, 176, 336), (672, 0, W))
    MM_CIX = (0, 1, 1, 2)



    def map_matmuls(j, pt):
        q, b = j % 4, j // 4
        for (dst0, s0, s1), cix in zip(MM_COLS, MM_CIX):
            lhsT = FYq[q][32 * q:32 * q + 3, b, :, cix::3]
            rhs = FXq[q][32 * q:32 * q + 3, b, :, s0:s1]
            nc.tensor.matmul(pt[:, dst0:dst0 + (s1 - s0)], lhsT, rhs,
                             start=True, stop=True,
                             perf_mode=mybir.MatmulPerfMode.DoubleRow,
                             tile_position=(32 * q, 0))

    # 4-map DMA groups, except the last four maps go in pairs so the
    # final transfer only trails the last drain by half a group
    group_starts = [(j0, GROUP) for j0 in range(0, NMAPS - GROUP, GROUP)]
    group_starts += [(NMAPS - GROUP, 2), (NMAPS - 2, 2)]
    for j0, gsz in group_starts:
        ug = ustage.tile([P, gsz, NW], FP16, tag="ug")
        for j in range(j0, j0 + gsz):
            g = j - j0
            pt = pmap.tile([P, 1024], F32, tag="pmap")  # 2 PSUM banks
            map_matmuls(j, pt)
            # scaled drain split across ACT and DVE; applies the host
            # peak scale (which folds in the fp8 gain compensation)
            nc.scalar.mul(ug[:, g, 0:XS], pt[:, 0:XS], RGC[:, j:j + 1])
            nc.vector.tensor_scalar_mul(ug[:, g, XS:NW], pt[:, XS:NW],
                                        RGC[:, j:j + 1])
        nc.sync.dma_start(out_t.ap()[:, j0:j0 + gsz, :], ug[:])


@functools.lru_cache(maxsize=1)
def _build():
    nc = bacc.Bacc("TRN2", target_bir_lowering=False, debug=False)
    fpl_in = nc.dram_tensor("fpl", [3, 2 * NR, W], FP8, kind="ExternalInput")
    rgc_in = nc.dram_tensor("rgc", [P, NMAPS], F32, kind="ExternalInput")
    out_t = nc.dram_tensor("out", [P, NMAPS, NW], FP16,
                           kind="ExternalOutput")
    with tile.TileContext(nc) as tc, ExitStack() as ctx:
        _emit(nc, tc, ctx, fpl_in, rgc_in, out_t)
    nc.compile()
    return nc


def _host_peaks(fyn, fxn, cx, cy, amp):
    """Peak of each map's 2-Gaussian mixture on the grid.

    fyn/fxn: [NR, W] float64 normalized factors, rows 2j/2j+1 = the two
    sets of map j; amp[r] = 1/((Sy_r+d)(Sx_r+d)) the component amplitude.
    The equal-sigma mixture's maximum lies on the segment between the
    centers; Newton from 3 starts plus exact evaluation of a 3x3 grid
    neighborhood around each candidate bounds the underestimate below
    2e-5 (validated against brute force).
    """
    cyx = np.stack([cx, cy], axis=1)  # true centers (x, y)
    c0, c1 = cyx[0::2], cyx[1::2]  # [NMAPS, 2] (x, y)
    A, B = amp[0::2], amp[1::2]
    d = np.linalg.norm(c1 - c0, axis=1)
    s2 = SIGMA * SIGMA

    t = np.stack([0.05 * d, 0.5 * d, 0.95 * d], 0)
    dd = d[None, :]
    An, Bn = A[None, :], B[None, :]
    for _ in range(30):
        e1 = An * np.exp(-t * t / (2 * s2))
        e2 = Bn * np.exp(-((dd - t) ** 2) / (2 * s2))
        g1 = -t / s2 * e1 + (dd - t) / s2 * e2
        h1 = (t * t / s2 - 1) / s2 * e1 + ((dd - t) ** 2 / s2 - 1) / s2 * e2
        step = np.where(np.abs(h1) > 1e-30, g1 / h1, 0.0)
        t = np.clip(t - step, 0.0, dd)

    dirv = c1 - c0
    with np.errstate(invalid="ignore", divide="ignore"):
        u = np.where(dd[:, :, None] > 1e-12,
                     dirv[None] * (t[:, :, None] / dd[:, :, None]), 0.0)
    pts = np.concatenate([c0[None] + u, c0[None], c1[None]], axis=0)

    fy0, fy1 = fyn[0::2], fyn[1::2]
    fx0, fx1 = fxn[0::2], fxn[1::2]
    r = np.arange(NMAPS)
    best = np.zeros(NMAPS)
    for k in range(pts.shape[0]):
        jx0 = np.clip(np.round(pts[k, :, 0] * (W - 1)).astype(int), 0, W - 1)
        jy0 = np.clip(np.round(pts[k, :, 1] * (W - 1)).astype(int), 0, W - 1)
        for dy in (-1, 0, 1):
            for dx in (-1, 0, 1):
                jx = np.clip(jx0 + dx, 0, W - 1)
                jy = np.clip(jy0 + dy, 0, W - 1)
                val = fy0[r, jy] * fx0[r, jx] + fy1[r, jy] * fx1[r, jx]
                best = np.maximum(best, val)
    return best


def _in_map_for(gaze, hand, b):
    cg = np.asarray(gaze[b], dtype=np.float32).reshape(NMAPS, 2)
    ch = np.asarray(hand[b], dtype=np.float32).reshape(NMAPS, 2)
    inter = np.stack([cg, ch], axis=1).reshape(NR, 2)  # row 2*j + t
    cx = inter[:, 0].astype(np.float64)
    cy = inter[:, 1].astype(np.float64)
    invalid = ~(cx + cy > 0)
    cy[invalid] = -10.0  # kills the y factor -> zero component

    fy = np.exp(EXP_SCALE * (_GRID64[None, :] - cy[:, None]) ** 2)
    fx = np.exp(EXP_SCALE * (_GRID64[None, :] - cx[:, None]) ** 2)
    sy = fy.sum(axis=1) + DELTA
    sx = fx.sum(axis=1) + DELTA
    fyn = fy / sy[:, None]
    fxn = fx / sx[:, None]
    amp = 1.0 / (sy * sx)

    yg = (fyn * Y_GAIN).astype(np.float32)
    xg = (fxn * X_GAIN).astype(np.float32)
    yh = yg.astype(F8NP)
    yl = (yg - yh.astype(np.float32)).astype(F8NP)
    xh = xg.astype(F8NP)
    xl = (xg - xh.astype(np.float32)).astype(F8NP)

    fpl = np.empty((3, 2 * NR, W), dtype=F8NP)
    fpl[0, 0:NR], fpl[0, NR:] = yh, xh
    fpl[1, 0:NR], fpl[1, NR:] = yh, xl
    fpl[2, 0:NR], fpl[2, NR:] = yl, xh

    rg = (DRAIN_SCALE / (_host_peaks(fyn, fxn, cx, cy, amp)
                         + EPS)).astype(np.float32)
    rgc = np.tile(rg[None, :], (P, 1))
    return {"fpl": fpl, "rgc": rgc}


def kernel(gaze_coords, hand_coords, _trace=False, **trace_kwargs):
    gaze_coords = np.asarray(gaze_coords, dtype=np.float32)
    hand_coords = np.asarray(hand_coords, dtype=np.float32)
    B = gaze_coords.shape[0]
    assert B == N_CORES, f"expected batch {N_CORES}, got {B}"
    nc = _build()
    in_maps = [_in_map_for(gaze_coords, hand_coords, b) for b in range(B)]
    res = run_bass_kernel_spmd(nc, in_maps, list(range(N_CORES)),
                               trace=_trace, **trace_kwargs)
    # device layout [112, 32, 1008] fp16 -> [32, 336, 336] fp32 per core
    outs = []
    for i in range(B):
        arr = np.asarray(res.results[i]["out"]).astype(np.float32)
        arr = arr.reshape(P, NMAPS, NCH, W).transpose(1, 0, 2, 3)
        outs.append(arr.reshape(NMAPS, H, W))
    out = np.stack(outs, axis=0).reshape(B, S_DIM, C_DIM, H, W)
    if _trace:
        return out, res
    return out
